# revision 32
# baseline (speedup 1.0000x reference)
"""2-layer GCN (GCNConv x2 + relu) on 8 TRN2 NeuronCores.

Distribution: nodes dst-sharded across 8 cores (12500 each). Since GCN has
no nonlinearity between the two convolutions, A(A(xW1)W2) = A(A(x W1W2)):
the dense transform y = x @ (W1@W2) is computed once (redundantly on every
core, rows pre-scaled by dinv on host), then TWO aggregation passes over
the same edge list. One AllGather (of the intermediate table) between them,
issued in 4 chunks so it overlaps the tail of pass 1.

Aggregation (per layer): messages table[src] are gathered row-wise from an
HBM table with the Q7 dma_gather (int16 indices -> 4 src chunks), spread
across the 4 SWDGE queues (one per chunk) so all four Q7 core-pairs
generate descriptors concurrently. Blocks of 128 edges are combined on the
TensorEngine with a per-block one-hot selector
  S[e, v] = (dstrel[e] == v)
built in ONE batched DVE is_equal per group (broadcast access patterns),
accumulating node-major windows in PSUM:
  agg[v, f] += sum_e S[e, v] * G[e, f]
dinv[dst] is applied at evacuation (per-window scale); dinv[src] is baked
into the table rows. Self-loops use a constant identity selector and
affine table reads (layer 1 reads the per-core ROTATED dense table so the
own shard sits at rows [0, SH)).
"""

import os

import numpy as np
import ml_dtypes

import concourse.bacc as bacc
import concourse.mybir as mybir
from concourse.tile import TileContext
from concourse.vector_clock import VectorClock, ScopedClock
from concourse import bass_utils

BF16 = ml_dtypes.bfloat16

# ---------------------------------------------------------------------------
# TileContext drain patch: this walrus rejects >1 sync wait on a TPB_CTRL
# Drain, so split the final drain into chained single-wait drains.
# ---------------------------------------------------------------------------


def _drain_and_barrier(self, tick_clock, wait_clock):
    gc = tick_clock.global_clock
    n = len(gc)
    procs = [p for p in range(n) if gc[p] > 0]
    chunks = [procs[i : i + 1] for i in range(len(procs))] or [[]]
    for chunk in chunks:
        vc = VectorClock([gc[p] if p in chunk else 0 for p in range(n)])
        drain_inst = self.nc.sync.drain()
        wait_clock.add_sem_waits(drain_inst.ins, ScopedClock({None: vc}))
    self.nc.all_engine_barrier()
    assert self.sems is not None
    popped = self.nc._tile_sem_poison_stack.pop()
    assert popped is self._sem_poison
    self.nc.clear_and_free_semaphores(list(self.sems.allocated().values()))
    self.nc.all_engine_barrier()


TileContext._drain_and_barrier = _drain_and_barrier


# ---------------------------------------------------------------------------
# Problem geometry (hardcoded for N=100000, F=C=128, 8 cores)
# ---------------------------------------------------------------------------

N_CORES = 8
N = 100000
SH = N // N_CORES            # 12500 nodes per shard
W = (SH + 127) // 128        # 98 dst windows per shard (last partial: 84)
GRP = 4                      # windows per group (psum tile)
NGRP = (W + GRP - 1) // GRP  # 25 groups (last group has 2 windows)
NCH = 4                      # gather chunks == SWDGE queues
CS1 = 25088                  # L1 chunk rows (196 tiles of 128; int16-safe)
NT2 = (NCH * CS1) // 128     # 784 dense tiles (100352 rows, padded)
# AllGather window split: chunk k covers windows [AGW[k], AGW[k+1]).
# Near-even split keeps per-(window,chunk) bins balanced (minimizes R).
AGW = [0, 24, 48, 72, 98]
AG_ROWS = [min((AGW[k + 1]) * 128, SH) - AGW[k] * 128 for k in range(4)]
XCH = 7                      # dense tiles per psum/evac chunk (divides 196)
XWR = 14                     # dense tiles per DMA write group (divides 196)
SCH_ROWS = XWR * 128         # 1792 rows per dense write group
# L1 table rows are PERMUTED within each write group (position s*1792+p*14+t
# holds rotated row s*1792 + t*128 + p) so the dense write is contiguous.


# ---------------------------------------------------------------------------
# Host-side graph preprocessing
# ---------------------------------------------------------------------------


def _edge_arrays(src_idx, chunk, dst_rel, R):
    """Build idx_wire / dr for one core and one layer.

    src_idx: per-edge index within its chunk's table.
    chunk:   per-edge chunk id (0..NCH-1).
    dst_rel: per-edge dst id relative to the shard (0..SH).
    Layout: groups g of GRP windows; within a group, blocks are ordered
    (ci, wi_rel, b) with exactly R blocks per (window, chunk) bin. The idx
    wire for gather call (g, ci) covers that call's nw*R blocks.
    Returns idx_wire [128, total_idx_cols] int16, dr [128, n_blocks] f32,
    and per-group idx column offsets.
    """
    w = dst_rel // 128
    order = np.lexsort((src_idx, chunk, w))
    s2 = src_idx[order]
    c2 = chunk[order]
    w2 = w[order]
    key2 = w2 * NCH + c2
    starts = np.searchsorted(key2, np.arange(W * NCH))
    ends = np.searchsorted(key2, np.arange(W * NCH) + 1)
    d2 = dst_rel[order]

    n_blocks = W * NCH * R
    total_idx_cols = n_blocks * 8
    idx_wire = np.zeros((128, total_idx_cols), np.int16)
    dr = np.full((128, n_blocks), -1.0, np.float32)
    grp_col_off = []

    col0 = 0
    blk0 = 0
    for g in range(NGRP):
        wlo = g * GRP
        whi = min(wlo + GRP, W)
        nw = whi - wlo
        grp_col_off.append(col0)
        for ci in range(NCH):
            # blocks for (g, ci): nw*R, idx cols nw*R*8
            for wi in range(wlo, whi):
                k = wi * NCH + ci
                a, b = int(starts[k]), int(ends[k])
                n = b - a
                assert n <= R * 128, f"bin overflow {n} > {R * 128}"
                # block index within group: (ci*nw + (wi-wlo))*R + b
                bw0 = blk0 + (ci * nw + (wi - wlo)) * R
                j = np.arange(n)
                p = j % 128
                bb = bw0 + j // 128
                dr[p, bb] = (d2[a:b] - wi * 128).astype(np.float32)
                # idx wire position: within gather call (g, ci), flat slot
                # jj = (wi-wlo)*R*128 + j, col = col0 + ci*nw*R*8 + jj//16
                jj = (wi - wlo) * R * 128 + j
                col = col0 + jj // 16
                row = jj % 16
                ss = s2[a:b].astype(np.int16)
                for rep in range(8):
                    idx_wire[rep * 16 + row, col] = ss
            col0 += nw * R * 8
        blk0 += NCH * nw * R
    return idx_wire, dr, grp_col_off, n_blocks, total_idx_cols


def _preprocess(x, edge_index, W1, b1, W2, b2):
    src_e = edge_index[0].astype(np.int64)
    dst_e = edge_index[1].astype(np.int64)

    deg = np.bincount(
        np.concatenate([dst_e, np.arange(N, dtype=np.int64)]), minlength=N
    ).astype(np.float64)
    dinv64 = 1.0 / np.sqrt(deg)
    dinv = dinv64.astype(np.float32)

    W12 = (np.asarray(W1, np.float64) @ np.asarray(W2, np.float64)).astype(BF16)
    b1W2 = (np.asarray(b1, np.float64) @ np.asarray(W2, np.float64)).astype(
        np.float64
    )
    has_b = bool(np.any(np.asarray(b1)) or np.any(np.asarray(b2)))
    # rowsum of A (incl self loop) for the b1 correction term
    if has_b:
        acc = np.zeros(N, np.float64)
        np.add.at(acc, dst_e, dinv64[src_e])
        rowsumA = dinv64 * (acc + dinv64)

    iota = np.tile(np.arange(128, dtype=np.float32).astype(BF16), (128, 1))
    iden = np.eye(128, dtype=np.float32).astype(BF16)

    # compute uniform R across cores and layers
    per_core = []
    for i in range(N_CORES):
        sel = (dst_e // SH) == i
        s = src_e[sel]
        d = dst_e[sel] - i * SH
        per_core.append((s, d))
    R = 1
    ag_off = np.array([AGW[0], AGW[1], AGW[2], AGW[3]], np.int64) * 128
    for i in range(N_CORES):
        s, d = per_core[i]
        w = d // 128
        # L1: rotated chunks
        rs = (s - i * SH) % N
        c1 = rs // CS1
        cnt = np.bincount(w * NCH + c1, minlength=W * NCH)
        R = max(R, int((cnt.max() + 127) // 128))
        # L2: AG slice chunks
        r = s % SH
        c2 = np.digitize(r, ag_off[1:])
        cnt = np.bincount(w * NCH + c2, minlength=W * NCH)
        R = max(R, int((cnt.max() + 127) // 128))

    x_sc = np.asarray(x, np.float64) * dinv64[:, None]  # dinv[src] prescale
    x_bf = x_sc.astype(BF16)

    in_maps = []
    meta = None
    for i in range(N_CORES):
        s, d = per_core[i]
        rs = (s - i * SH) % N
        c1 = rs // CS1
        loc = rs - c1 * CS1
        sc = loc // SCH_ROWS
        rem = loc % SCH_ROWS
        i1 = sc * SCH_ROWS + (rem % 128) * XWR + rem // 128  # permuted pos
        idx1, dr1, goff, n_blocks, idx_cols = _edge_arrays(i1, c1, d, R)

        r = s % SH
        c2 = np.digitize(r, ag_off[1:])
        rows_k = np.array(AG_ROWS, np.int64)
        i2 = (s // SH) * rows_k[c2] + (r - ag_off[c2])
        idx2, dr2, goff2, n_blocks2, idx_cols2 = _edge_arrays(i2, c2, d, R)
        assert goff == goff2 and n_blocks == n_blocks2 and idx_cols == idx_cols2

        # rotated, dinv-prescaled x, feature-major, padded to NT2*128 rows
        x_rot = np.zeros((128, NT2 * 128), BF16)
        x_rot[:, :N] = np.roll(x_bf, -i * SH, axis=0).T

        flat = dinv[i * SH : (i + 1) * SH]
        dwt = np.zeros((128, W), np.float32)
        for wi in range(W):
            nn = min(128, SH - wi * 128)
            dwt[:nn, wi] = flat[wi * 128 : wi * 128 + nn]
        dw2 = dwt * dwt

        im = {
            "x_fm": x_rot, "W12": W12, "iota": iota, "iden": iden,
            "dwt": dwt, "dw2": dw2,
            "idx1": idx1, "dr1": dr1.astype(BF16),
            "idx2": idx2, "dr2": dr2.astype(BF16),
        }
        if has_b:
            # L2 psum correction: two rank-1 terms, pre-divided by dinv[v]
            lhs = np.zeros((2, W * 128), np.float32)
            lhs[0, :SH] = (rowsumA / dinv64)[i * SH : (i + 1) * SH]
            lhs[1, :SH] = (1.0 / dinv64)[i * SH : (i + 1) * SH]
            rhs = np.zeros((2, 128), np.float32)
            rhs[0] = b1W2
            rhs[1] = np.asarray(b2, np.float64)
            im["corr_lhs"] = lhs.astype(BF16)
            im["corr_rhs"] = rhs.astype(BF16)
        in_maps.append(im)
        meta = dict(R=R, n_blocks=n_blocks, idx_cols=idx_cols, goff=goff,
                    has_b=has_b)
    return meta, in_maps


# ---------------------------------------------------------------------------
# Bass kernel builder
# ---------------------------------------------------------------------------


def _build(meta):
    R = meta["R"]
    n_blocks = meta["n_blocks"]
    idx_cols = meta["idx_cols"]
    goff = meta["goff"]
    has_b = meta["has_b"]
    dt = mybir.dt

    nc = bacc.Bacc("TRN2", target_bir_lowering=False, debug=False,
                   num_swdge_queues=NCH)

    def inp(name, shape, dtype):
        return nc.dram_tensor(name, shape, dtype, kind="ExternalInput")

    x_fm = inp("x_fm", [128, NT2 * 128], dt.bfloat16)
    W12 = inp("W12", [128, 128], dt.bfloat16)
    iota_d = inp("iota", [128, 128], dt.bfloat16)
    iden_d = inp("iden", [128, 128], dt.bfloat16)
    dwt_d = inp("dwt", [128, W], dt.float32)
    dw2_d = inp("dw2", [128, W], dt.float32)
    idx_d = [inp("idx1", [128, idx_cols], dt.int16),
             inp("idx2", [128, idx_cols], dt.int16)]
    dr_d = [inp("dr1", [128, n_blocks], dt.bfloat16),
            inp("dr2", [128, n_blocks], dt.bfloat16)]
    if has_b:
        corr_lhs = inp("corr_lhs", [2, W * 128], dt.bfloat16)
        corr_rhs = inp("corr_rhs", [2, 128], dt.bfloat16)

    h1s_c = [nc.dram_tensor(f"h1s_c{k}", [CS1, 128], dt.bfloat16)
             for k in range(NCH)]
    t2sh = [nc.dram_tensor(f"t2sh{k}", [AG_ROWS[k], 128], dt.bfloat16)
            for k in range(NCH)]
    t2f = [nc.dram_tensor(f"t2f{k}", [N_CORES * AG_ROWS[k], 128], dt.bfloat16,
                          addr_space="Shared")
           for k in range(NCH)]
    # local (non-Shared) copies of the AllGather outputs: gather-descriptor
    # reads from Shared-space HBM drain ~40% slower, so copy once after each
    # collective and gather from the local tensors
    t2l = [nc.dram_tensor(f"t2l{k}", [N_CORES * AG_ROWS[k], 128], dt.bfloat16)
           for k in range(NCH)]
    out_d = nc.dram_tensor("out", [SH, 128], dt.float32, kind="ExternalOutput")

    with TileContext(nc) as tc:
        with (
            tc.tile_pool(name="const", bufs=1) as constp,
            tc.tile_pool(name="selfr", bufs=2) as selfrp,
            tc.tile_pool(name="corrp", bufs=1) as corrp,
            tc.tile_pool(name="xs", bufs=3) as xs,
            tc.tile_pool(name="hstage", bufs=3) as hstage,
            tc.tile_pool(name="idxg", bufs=3) as idxgp,
            tc.tile_pool(name="drg", bufs=3) as drgp,
            tc.tile_pool(name="mask", bufs=2) as maskp,
            tc.tile_pool(name="gbuf", bufs=3) as gbufp,
            tc.tile_pool(name="zst", bufs=3) as zstp,
            tc.tile_pool(name="outst", bufs=3) as outstp,
            tc.tile_pool(name="psA", bufs=3, space="PSUM") as psA,
            tc.tile_pool(name="psD", bufs=2, space="PSUM") as psD,
        ):
            w12t = constp.tile([128, 128], dt.bfloat16)
            nc.sync.dma_start(w12t[:], W12[:])
            iot = constp.tile([128, 128], dt.bfloat16)
            nc.sync.dma_start(iot[:], iota_d[:])
            idt = constp.tile([128, 128], dt.bfloat16)
            nc.sync.dma_start(idt[:], iden_d[:])
            dwt = constp.tile([128, W], dt.float32)
            nc.sync.dma_start(dwt[:], dwt_d[:])
            dw2 = constp.tile([128, W], dt.float32)
            nc.sync.dma_start(dw2[:], dw2_d[:])
            if has_b:
                clh = corrp.tile([2, W * 128], dt.bfloat16)
                nc.sync.dma_start(clh[:], corr_lhs[:])
                crh = corrp.tile([2, 128], dt.bfloat16)
                nc.sync.dma_start(crh[:], corr_rhs[:])

            # ------------- dense: h1s = (dinv*x) @ W12 (rotated order) ----
            # h1s rows are permuted within each superchunk (row s*896+p*7+t
            # holds node s*896+t*128+p) so this write is fully contiguous.
            # L1 self rows (windows of the own shard = first 98 tiles, all in
            # chunk 0) are also staged into SBUF straight from PSUM.
            selfrows1 = selfrp.tile([128, W, 128], dt.bfloat16,
                                    tag="selfrows")
            for pg in range(NT2 // XWR):
                t0 = pg * XWR
                xt = xs.tile([128, XWR * 128], dt.bfloat16, tag="xt")
                nc.sync.dma_start(
                    xt[:], x_fm[:, t0 * 128 : (t0 + XWR) * 128]
                )
                hst = hstage.tile([128, XWR, 128], dt.bfloat16, tag="hst")
                for half in range(2):
                    h0 = half * XCH
                    ps = psD.tile([128, XCH, 128], dt.float32, tag="pd")
                    for t in range(XCH):
                        nc.tensor.matmul(
                            ps[:, t, :],
                            xt[:, (h0 + t) * 128 : (h0 + t + 1) * 128],
                            w12t[:],
                            start=True, stop=True,
                        )
                    # alternate evacuation between Scalar and DVE so
                    # neither engine paces the dense phase
                    if half == 0:
                        nc.scalar.activation(
                            hst[:, h0 : h0 + XCH, :].rearrange(
                                "p t f -> p (t f)"),
                            ps[:].rearrange("p t f -> p (t f)"),
                            mybir.ActivationFunctionType.Copy, scale=1.0,
                        )
                    else:
                        nc.vector.tensor_copy(
                            hst[:, h0 : h0 + XCH, :].rearrange(
                                "p t f -> p (t f)"),
                            ps[:].rearrange("p t f -> p (t f)"),
                        )
                    if t0 + h0 < W:
                        nw = min(XCH, W - t0 - h0)
                        nc.vector.tensor_copy(
                            selfrows1[:, t0 + h0 : t0 + h0 + nw, :].rearrange(
                                "p t f -> p (t f)"),
                            ps[:, :nw, :].rearrange("p t f -> p (t f)"),
                        )
                ck = t0 // (CS1 // 128)
                s_in = (t0 % (CS1 // 128)) // XWR
                nc.scalar.dma_start(
                    h1s_c[ck][s_in * SCH_ROWS : (s_in + 1) * SCH_ROWS, :]
                    .rearrange("(p t) f -> p t f", p=128),
                    hst[:],
                )

            # ------------- aggregation (layer = 0 or 1) ------------------
            def issue_ag(k, selfrows_next):
                nc.gpsimd.collective_compute(
                    "AllGather",
                    mybir.AluOpType.bypass,
                    ins=[t2sh[k][:]],
                    outs=[t2f[k][:]],
                    replica_groups=[list(range(N_CORES))],
                )
                nc.sync.dma_start(t2l[k][:], t2f[k][:])
                # stage this chunk's own-shard rows for the next layer's
                # self blocks while layer-1 still runs
                wlo = AGW[k]
                full = (AG_ROWS[k] // 128) * 128
                nc.scalar.dma_start(
                    selfrows_next[:, wlo : wlo + full // 128, :],
                    t2sh[k][:full, :].rearrange("(w p) f -> p w f", p=128),
                )
                if AG_ROWS[k] > full:
                    rem = AG_ROWS[k] - full
                    nc.scalar.dma_start(
                        selfrows_next[:rem, wlo + full // 128, :],
                        t2sh[k][full:, :],
                    )

            def agg_layer(layer, tables, selfrows, selfrows_next=None):
                pending_ag = []
                for g in range(NGRP):
                    wlo = g * GRP
                    whi = min(wlo + GRP, W)
                    nw = whi - wlo
                    nblk = nw * R          # blocks per gather call
                    gblk = NCH * nblk      # blocks per group
                    blk0 = wlo * NCH * R   # first block of group

                    drt = drgp.tile([128, GRP * NCH * R], dt.bfloat16,
                                    tag="drt")
                    nc.sync.dma_start(
                        drt[:, :gblk], dr_d[layer][:, blk0 : blk0 + gblk]
                    )
                    stw = maskp.tile([128, GRP * NCH * R, 128], dt.bfloat16,
                                     tag="stw")
                    nc.vector.tensor_tensor(
                        stw[:, :gblk, :],
                        iot[:].rearrange("p (o v) -> p o v", o=1)
                              .to_broadcast([128, gblk, 128]),
                        drt[:, :gblk].rearrange("p (b o) -> p b o", o=1)
                                     .to_broadcast([128, gblk, 128]),
                        mybir.AluOpType.is_equal,
                    )

                    gts = []
                    for ci in range(NCH):
                        ixt = idxgp.tile([128, GRP * R * 8], dt.int16,
                                         tag=f"ix{ci}")
                        c0 = goff[g] + ci * nblk * 8
                        nc.sync.dma_start(
                            ixt[:, : nblk * 8],
                            idx_d[layer][:, c0 : c0 + nblk * 8],
                        )
                        gt = gbufp.tile([128, GRP * R, 128], dt.bfloat16,
                                        tag=f"gt{ci}")
                        nc.gpsimd.dma_gather(
                            gt[:, :nblk, :],
                            tables[ci][:],
                            ixt[:, : nblk * 8],
                            num_idxs=nblk * 128,
                            num_idxs_reg=nblk * 128,
                            elem_size=128,
                            elem_step=128,
                            single_packet=False,
                            queue_num=ci,
                        )
                        gts.append(gt)

                    # issue any pending AllGather AFTER this group's gathers
                    # so the gpsimd engine stall (waiting on t2sh writes)
                    # doesn't delay them
                    for k in pending_ag:
                        issue_ag(k, selfrows_next)
                    pending_ag = []

                    psg = psA.tile([128, GRP, 128], dt.float32, tag="psg")
                    for wi in range(wlo, whi):
                        wr = wi - wlo
                        nn = min(128, SH - wi * 128)
                        for ci in range(NCH):
                            for b in range(R):
                                blk = (ci * nw + wr) * R + b
                                nc.tensor.matmul(
                                    psg[:, wr, :],
                                    stw[:, blk, :],
                                    gts[ci][:, wr * R + b, :],
                                    start=(ci == 0 and b == 0),
                                    stop=False,
                                )
                        if has_b and layer == 1:
                            nc.tensor.matmul(
                                psg[:, wr, :],
                                clh[:, wi * 128 : (wi + 1) * 128],
                                crh[:],
                                start=False, stop=False,
                            )
                        nc.tensor.matmul(
                            psg[:, wr, :],
                            idt[:nn, :],
                            selfrows[:nn, wi, :],
                            start=False, stop=True,
                        )

                    if layer == 0:
                        # table2 rows = dinv^2 * psum, bf16, window-sharded
                        zt = zstp.tile([128, GRP, 128], dt.bfloat16, tag="zt")
                        nc.vector.tensor_tensor(
                            zt[:, :nw, :],
                            psg[:, :nw, :],
                            dw2[:, wlo:whi].rearrange("p (b o) -> p b o", o=1)
                                           .to_broadcast([128, nw, 128]),
                            mybir.AluOpType.mult,
                        )
                        for k in range(NCH):
                            lo = max(wlo, AGW[k])
                            hi = min(whi, AGW[k + 1])
                            if lo >= hi:
                                continue
                            full = AGW[k] * 128 + AG_ROWS[k]
                            r0 = lo * 128 - AGW[k] * 128
                            r1 = min(hi * 128, full) - AGW[k] * 128
                            nwk = (r1 - r0 + 127) // 128
                            wfull = (r1 - r0) // 128
                            if wfull:
                                nc.sync.dma_start(
                                    t2sh[k][r0 : r0 + wfull * 128, :]
                                    .rearrange("(w p) f -> p w f", p=128),
                                    zt[:, lo - wlo : lo - wlo + wfull, :],
                                )
                            if nwk > wfull:
                                rem = (r1 - r0) - wfull * 128
                                nc.sync.dma_start(
                                    t2sh[k][r0 + wfull * 128 : r1, :],
                                    zt[:rem, lo - wlo + wfull, :],
                                )
                        # AllGather chunk as soon as its windows are done
                        for k in range(NCH):
                            if whi == AGW[k + 1]:
                                pending_ag.append(k)
                    else:
                        for wi in range(wlo, whi):
                            wr = wi - wlo
                            nn = min(128, SH - wi * 128)
                            ot = outstp.tile([128, 128], dt.float32, tag="ot")
                            nc.scalar.activation(
                                ot[:], psg[:, wr, :],
                                mybir.ActivationFunctionType.Relu,
                                scale=dwt[:, wi : wi + 1],
                            )
                            nc.sync.dma_start(
                                out_d[wi * 128 : wi * 128 + nn, :], ot[:nn, :]
                            )
                for k in pending_ag:
                    issue_ag(k, selfrows_next)

            # L2 self rows tile is filled chunk-by-chunk during layer 1
            selfrows2 = selfrp.tile([128, W, 128], dt.bfloat16,
                                    tag="selfrows")
            agg_layer(0, h1s_c, selfrows1, selfrows_next=selfrows2)
            agg_layer(1, t2l, selfrows2)

    nc.compile()
    return nc


def kernel(x, edge_index, W1, b1, W2, b2):
    x = np.asarray(x)
    meta, in_maps = _preprocess(
        x, np.asarray(edge_index), np.asarray(W1), np.asarray(b1),
        np.asarray(W2), np.asarray(b2),
    )
    nc = _build(meta)
    trace = bool(os.environ.get("KERNEL_TRACE"))
    res = bass_utils.run_bass_kernel_spmd(
        nc, in_maps, core_ids=list(range(N_CORES)), trace=trace
    )
    global last_exec_time_ns
    last_exec_time_ns = res.exec_time_ns
    out = np.empty((N, 128), np.float32)
    for i in range(N_CORES):
        out[i * SH : (i + 1) * SH, :] = res.results[i]["out"]
    return out


# revision 36
# speedup vs baseline: 1.0342x; 1.0342x over previous
"""2-layer GCN (GCNConv x2 + relu) on 8 TRN2 NeuronCores.

Distribution: nodes dst-sharded across 8 cores (12500 each). Since GCN has
no nonlinearity between the two convolutions, A(A(xW1)W2) = A(A(x W1W2)):
the dense transform y = x @ (W1@W2) is computed once (redundantly on every
core, rows pre-scaled by dinv on host), then TWO aggregation passes over
the same edge list. One AllGather (of the intermediate table) between them,
issued in 4 chunks so it overlaps the tail of pass 1.

Aggregation (per layer): messages table[src] are gathered row-wise from an
HBM table with the Q7 dma_gather (int16 indices -> 4 src chunks), spread
across the 4 SWDGE queues (one per chunk) so all four Q7 core-pairs
generate descriptors concurrently. Blocks of 128 edges are combined on the
TensorEngine with a per-block one-hot selector
  S[e, v] = (dstrel[e] == v)
built in ONE batched DVE is_equal per group (broadcast access patterns),
accumulating node-major windows in PSUM:
  agg[v, f] += sum_e S[e, v] * G[e, f]
dinv[dst] is applied at evacuation (per-window scale); dinv[src] is baked
into the table rows. Self-loops use a constant identity selector and
affine table reads (layer 1 reads the per-core ROTATED dense table so the
own shard sits at rows [0, SH)).
"""

import os

import numpy as np
import ml_dtypes

import concourse.bacc as bacc
import concourse.mybir as mybir
from concourse.tile import TileContext
from concourse.vector_clock import VectorClock, ScopedClock
from concourse import bass_utils

BF16 = ml_dtypes.bfloat16

# ---------------------------------------------------------------------------
# TileContext drain patch: this walrus rejects >1 sync wait on a TPB_CTRL
# Drain, so split the final drain into chained single-wait drains.
# ---------------------------------------------------------------------------


def _drain_and_barrier(self, tick_clock, wait_clock):
    gc = tick_clock.global_clock
    n = len(gc)
    procs = [p for p in range(n) if gc[p] > 0]
    chunks = [procs[i : i + 1] for i in range(len(procs))] or [[]]
    for chunk in chunks:
        vc = VectorClock([gc[p] if p in chunk else 0 for p in range(n)])
        drain_inst = self.nc.sync.drain()
        wait_clock.add_sem_waits(drain_inst.ins, ScopedClock({None: vc}))
    self.nc.all_engine_barrier()
    assert self.sems is not None
    popped = self.nc._tile_sem_poison_stack.pop()
    assert popped is self._sem_poison
    self.nc.clear_and_free_semaphores(list(self.sems.allocated().values()))
    self.nc.all_engine_barrier()


TileContext._drain_and_barrier = _drain_and_barrier


# ---------------------------------------------------------------------------
# Problem geometry (hardcoded for N=100000, F=C=128, 8 cores)
# ---------------------------------------------------------------------------

N_CORES = 8
N = 100000
SH = N // N_CORES            # 12500 nodes per shard
W = (SH + 127) // 128        # 98 dst windows per shard (last partial: 84)
GRP = 4                      # windows per group (psum tile)
NGRP = (W + GRP - 1) // GRP  # 25 groups (last group has 2 windows)
NCH = 4                      # gather chunks == SWDGE queues
CS1 = 25088                  # L1 chunk rows (196 tiles of 128; int16-safe)
NT2 = (NCH * CS1) // 128     # 784 dense tiles (100352 rows, padded)
# AllGather window split: chunk k covers windows [AGW[k], AGW[k+1]).
# Near-even split keeps per-(window,chunk) bins balanced (minimizes R).
AGW = [0, 24, 48, 72, 98]
AG_ROWS = [min((AGW[k + 1]) * 128, SH) - AGW[k] * 128 for k in range(4)]
XCH = 7                      # dense tiles per psum/evac chunk (divides 196)
XWR = 14                     # dense tiles per DMA write group (divides 196)
SCH_ROWS = XWR * 128         # 1792 rows per dense write group
# L1 table rows are PERMUTED within each write group (position s*1792+p*14+t
# holds rotated row s*1792 + t*128 + p) so the dense write is contiguous.


# ---------------------------------------------------------------------------
# Host-side graph preprocessing
# ---------------------------------------------------------------------------


def _edge_arrays(src_idx, chunk, dst_rel, R):
    """Build idx_wire / dr for one core and one layer.

    src_idx: per-edge index within its chunk's table.
    chunk:   per-edge chunk id (0..NCH-1).
    dst_rel: per-edge dst id relative to the shard (0..SH).
    Layout: groups g of GRP windows; within a group, blocks are ordered
    (ci, wi_rel, b) with exactly R blocks per (window, chunk) bin. The idx
    wire for gather call (g, ci) covers that call's nw*R blocks.
    Returns idx_wire [128, total_idx_cols] int16, dr [128, n_blocks] f32,
    and per-group idx column offsets.
    """
    w = dst_rel // 128
    order = np.lexsort((src_idx, chunk, w))
    s2 = src_idx[order]
    c2 = chunk[order]
    w2 = w[order]
    key2 = w2 * NCH + c2
    starts = np.searchsorted(key2, np.arange(W * NCH))
    ends = np.searchsorted(key2, np.arange(W * NCH) + 1)
    d2 = dst_rel[order]

    n_blocks = W * NCH * R
    total_idx_cols = n_blocks * 8
    idx_wire = np.zeros((128, total_idx_cols), np.int16)
    dr = np.full((128, n_blocks), -1.0, np.float32)
    grp_col_off = []

    col0 = 0
    blk0 = 0
    for g in range(NGRP):
        wlo = g * GRP
        whi = min(wlo + GRP, W)
        nw = whi - wlo
        grp_col_off.append(col0)
        for ci in range(NCH):
            # blocks for (g, ci): nw*R, idx cols nw*R*8
            for wi in range(wlo, whi):
                k = wi * NCH + ci
                a, b = int(starts[k]), int(ends[k])
                n = b - a
                assert n <= R * 128, f"bin overflow {n} > {R * 128}"
                # block index within group: (ci*nw + (wi-wlo))*R + b
                bw0 = blk0 + (ci * nw + (wi - wlo)) * R
                j = np.arange(n)
                p = j % 128
                bb = bw0 + j // 128
                dr[p, bb] = (d2[a:b] - wi * 128).astype(np.float32)
                # idx wire position: within gather call (g, ci), flat slot
                # jj = (wi-wlo)*R*128 + j, col = col0 + ci*nw*R*8 + jj//16
                jj = (wi - wlo) * R * 128 + j
                col = col0 + jj // 16
                row = jj % 16
                ss = s2[a:b].astype(np.int16)
                for rep in range(8):
                    idx_wire[rep * 16 + row, col] = ss
            col0 += nw * R * 8
        blk0 += NCH * nw * R
    return idx_wire, dr, grp_col_off, n_blocks, total_idx_cols


def _preprocess(x, edge_index, W1, b1, W2, b2):
    src_e = edge_index[0].astype(np.int64)
    dst_e = edge_index[1].astype(np.int64)

    deg = np.bincount(
        np.concatenate([dst_e, np.arange(N, dtype=np.int64)]), minlength=N
    ).astype(np.float64)
    dinv64 = 1.0 / np.sqrt(deg)
    dinv = dinv64.astype(np.float32)

    W12 = (np.asarray(W1, np.float64) @ np.asarray(W2, np.float64)).astype(BF16)
    b1W2 = (np.asarray(b1, np.float64) @ np.asarray(W2, np.float64)).astype(
        np.float64
    )
    has_b = bool(np.any(np.asarray(b1)) or np.any(np.asarray(b2)))
    # rowsum of A (incl self loop) for the b1 correction term
    if has_b:
        acc = np.zeros(N, np.float64)
        np.add.at(acc, dst_e, dinv64[src_e])
        rowsumA = dinv64 * (acc + dinv64)

    iota = np.tile(np.arange(128, dtype=np.float32).astype(BF16), (128, 1))
    iden = np.eye(128, dtype=np.float32).astype(BF16)

    # compute uniform R across cores and layers
    per_core = []
    for i in range(N_CORES):
        sel = (dst_e // SH) == i
        s = src_e[sel]
        d = dst_e[sel] - i * SH
        per_core.append((s, d))
    R = 1
    ag_off = np.array([AGW[0], AGW[1], AGW[2], AGW[3]], np.int64) * 128
    for i in range(N_CORES):
        s, d = per_core[i]
        w = d // 128
        # L1: rotated chunks
        rs = (s - i * SH) % N
        c1 = rs // CS1
        cnt = np.bincount(w * NCH + c1, minlength=W * NCH)
        R = max(R, int((cnt.max() + 127) // 128))
        # L2: AG slice chunks
        r = s % SH
        c2 = np.digitize(r, ag_off[1:])
        cnt = np.bincount(w * NCH + c2, minlength=W * NCH)
        R = max(R, int((cnt.max() + 127) // 128))

    x_sc = np.asarray(x, np.float64) * dinv64[:, None]  # dinv[src] prescale
    x_bf = x_sc.astype(BF16)

    in_maps = []
    meta = None
    for i in range(N_CORES):
        s, d = per_core[i]
        rs = (s - i * SH) % N
        c1 = rs // CS1
        loc = rs - c1 * CS1
        sc = loc // SCH_ROWS
        rem = loc % SCH_ROWS
        i1 = sc * SCH_ROWS + (rem % 128) * XWR + rem // 128  # permuted pos
        idx1, dr1, goff, n_blocks, idx_cols = _edge_arrays(i1, c1, d, R)

        r = s % SH
        c2 = np.digitize(r, ag_off[1:])
        rows_k = np.array(AG_ROWS, np.int64)
        i2 = (s // SH) * rows_k[c2] + (r - ag_off[c2])
        idx2, dr2, goff2, n_blocks2, idx_cols2 = _edge_arrays(i2, c2, d, R)
        assert goff == goff2 and n_blocks == n_blocks2 and idx_cols == idx_cols2

        # rotated, dinv-prescaled x, feature-major, padded to NT2*128 rows
        x_rot = np.zeros((128, NT2 * 128), BF16)
        x_rot[:, :N] = np.roll(x_bf, -i * SH, axis=0).T

        flat = dinv[i * SH : (i + 1) * SH]
        dwt = np.zeros((128, W), np.float32)
        for wi in range(W):
            nn = min(128, SH - wi * 128)
            dwt[:nn, wi] = flat[wi * 128 : wi * 128 + nn]
        dw2 = dwt * dwt

        im = {
            "x_fm": x_rot, "W12": W12, "iota": iota, "iden": iden,
            "dwt": dwt, "dw2": dw2,
            "idx1": idx1, "dr1": dr1.astype(BF16),
            "idx2": idx2, "dr2": dr2.astype(BF16),
        }
        if has_b:
            # L2 psum correction: two rank-1 terms, pre-divided by dinv[v]
            lhs = np.zeros((2, W * 128), np.float32)
            lhs[0, :SH] = (rowsumA / dinv64)[i * SH : (i + 1) * SH]
            lhs[1, :SH] = (1.0 / dinv64)[i * SH : (i + 1) * SH]
            rhs = np.zeros((2, 128), np.float32)
            rhs[0] = b1W2
            rhs[1] = np.asarray(b2, np.float64)
            im["corr_lhs"] = lhs.astype(BF16)
            im["corr_rhs"] = rhs.astype(BF16)
        in_maps.append(im)
        meta = dict(R=R, n_blocks=n_blocks, idx_cols=idx_cols, goff=goff,
                    has_b=has_b)
    return meta, in_maps


# ---------------------------------------------------------------------------
# Bass kernel builder
# ---------------------------------------------------------------------------


def _build(meta):
    R = meta["R"]
    n_blocks = meta["n_blocks"]
    idx_cols = meta["idx_cols"]
    goff = meta["goff"]
    has_b = meta["has_b"]
    dt = mybir.dt

    nc = bacc.Bacc("TRN2", target_bir_lowering=False, debug=False,
                   num_swdge_queues=NCH)

    def inp(name, shape, dtype):
        return nc.dram_tensor(name, shape, dtype, kind="ExternalInput")

    x_fm = inp("x_fm", [128, NT2 * 128], dt.bfloat16)
    W12 = inp("W12", [128, 128], dt.bfloat16)
    iota_d = inp("iota", [128, 128], dt.bfloat16)
    iden_d = inp("iden", [128, 128], dt.bfloat16)
    dwt_d = inp("dwt", [128, W], dt.float32)
    dw2_d = inp("dw2", [128, W], dt.float32)
    idx_d = [inp("idx1", [128, idx_cols], dt.int16),
             inp("idx2", [128, idx_cols], dt.int16)]
    dr_d = [inp("dr1", [128, n_blocks], dt.bfloat16),
            inp("dr2", [128, n_blocks], dt.bfloat16)]
    if has_b:
        corr_lhs = inp("corr_lhs", [2, W * 128], dt.bfloat16)
        corr_rhs = inp("corr_rhs", [2, 128], dt.bfloat16)

    h1s_c = [nc.dram_tensor(f"h1s_c{k}", [CS1, 128], dt.bfloat16)
             for k in range(NCH)]
    t2sh = [nc.dram_tensor(f"t2sh{k}", [AG_ROWS[k], 128], dt.bfloat16)
            for k in range(NCH)]
    t2f = [nc.dram_tensor(f"t2f{k}", [N_CORES * AG_ROWS[k], 128], dt.bfloat16,
                          addr_space="Shared")
           for k in range(NCH)]
    # local (non-Shared) copies of the AllGather outputs: gather-descriptor
    # reads from Shared-space HBM drain ~40% slower, so copy once after each
    # collective and gather from the local tensors
    t2l = [nc.dram_tensor(f"t2l{k}", [N_CORES * AG_ROWS[k], 128], dt.bfloat16)
           for k in range(NCH)]
    out_d = nc.dram_tensor("out", [SH, 128], dt.float32, kind="ExternalOutput")

    with TileContext(nc) as tc:
        with (
            tc.tile_pool(name="const", bufs=1) as constp,
            tc.tile_pool(name="selfr", bufs=2) as selfrp,
            tc.tile_pool(name="corrp", bufs=1) as corrp,
            tc.tile_pool(name="xs", bufs=3) as xs,
            tc.tile_pool(name="hstage", bufs=3) as hstage,
            tc.tile_pool(name="idxg", bufs=3) as idxgp,
            tc.tile_pool(name="drg", bufs=3) as drgp,
            tc.tile_pool(name="mask", bufs=2) as maskp,
            tc.tile_pool(name="gbuf", bufs=3) as gbufp,
            tc.tile_pool(name="zst", bufs=3) as zstp,
            tc.tile_pool(name="outst", bufs=3) as outstp,
            tc.tile_pool(name="psA", bufs=3, space="PSUM") as psA,
            tc.tile_pool(name="psD", bufs=2, space="PSUM") as psD,
        ):
            w12t = constp.tile([128, 128], dt.bfloat16)
            nc.sync.dma_start(w12t[:], W12[:])
            iot = constp.tile([128, 128], dt.bfloat16)
            nc.sync.dma_start(iot[:], iota_d[:])
            idt = constp.tile([128, 128], dt.bfloat16)
            nc.sync.dma_start(idt[:], iden_d[:])
            dwt = constp.tile([128, W], dt.float32)
            nc.sync.dma_start(dwt[:], dwt_d[:])
            dw2 = constp.tile([128, W], dt.float32)
            nc.sync.dma_start(dw2[:], dw2_d[:])
            if has_b:
                clh = corrp.tile([2, W * 128], dt.bfloat16)
                nc.sync.dma_start(clh[:], corr_lhs[:])
                crh = corrp.tile([2, 128], dt.bfloat16)
                nc.sync.dma_start(crh[:], corr_rhs[:])

            # ------------- dense: h1s = (dinv*x) @ W12 (rotated order) ----
            # h1s rows are permuted within each superchunk (row s*896+p*7+t
            # holds node s*896+t*128+p) so this write is fully contiguous.
            # L1 self rows (windows of the own shard = first 98 tiles, all in
            # chunk 0) are also staged into SBUF straight from PSUM.
            selfrows1 = selfrp.tile([128, W, 128], dt.bfloat16,
                                    tag="selfrows")
            for pg in range(NT2 // XWR):
                t0 = pg * XWR
                xt = xs.tile([128, XWR * 128], dt.bfloat16, tag="xt")
                nc.sync.dma_start(
                    xt[:], x_fm[:, t0 * 128 : (t0 + XWR) * 128]
                )
                hst = hstage.tile([128, XWR, 128], dt.bfloat16, tag="hst")
                for half in range(2):
                    h0 = half * XCH
                    ps = psD.tile([128, XCH, 128], dt.float32, tag="pd")
                    for t in range(XCH):
                        nc.tensor.matmul(
                            ps[:, t, :],
                            xt[:, (h0 + t) * 128 : (h0 + t + 1) * 128],
                            w12t[:],
                            start=True, stop=True,
                        )
                    # alternate evacuation between Scalar and DVE so
                    # neither engine paces the dense phase
                    if half == 0:
                        nc.scalar.activation(
                            hst[:, h0 : h0 + XCH, :].rearrange(
                                "p t f -> p (t f)"),
                            ps[:].rearrange("p t f -> p (t f)"),
                            mybir.ActivationFunctionType.Copy, scale=1.0,
                        )
                    else:
                        nc.vector.tensor_copy(
                            hst[:, h0 : h0 + XCH, :].rearrange(
                                "p t f -> p (t f)"),
                            ps[:].rearrange("p t f -> p (t f)"),
                        )
                    if t0 + h0 < W:
                        nw = min(XCH, W - t0 - h0)
                        nc.vector.tensor_copy(
                            selfrows1[:, t0 + h0 : t0 + h0 + nw, :].rearrange(
                                "p t f -> p (t f)"),
                            ps[:, :nw, :].rearrange("p t f -> p (t f)"),
                        )
                ck = t0 // (CS1 // 128)
                s_in = (t0 % (CS1 // 128)) // XWR
                nc.scalar.dma_start(
                    h1s_c[ck][s_in * SCH_ROWS : (s_in + 1) * SCH_ROWS, :]
                    .rearrange("(p t) f -> p t f", p=128),
                    hst[:],
                )

            # ------------- aggregation (layer = 0 or 1) ------------------
            def issue_ag(k, selfrows_next):
                nc.gpsimd.collective_compute(
                    "AllGather",
                    mybir.AluOpType.bypass,
                    ins=[t2sh[k][:]],
                    outs=[t2f[k][:]],
                    replica_groups=[list(range(N_CORES))],
                )
                # stage this chunk's own-shard rows for the next layer's
                # self blocks while layer-1 still runs
                wlo = AGW[k]
                full = (AG_ROWS[k] // 128) * 128
                nc.scalar.dma_start(
                    selfrows_next[:, wlo : wlo + full // 128, :],
                    t2sh[k][:full, :].rearrange("(w p) f -> p w f", p=128),
                )
                if AG_ROWS[k] > full:
                    rem = AG_ROWS[k] - full
                    nc.scalar.dma_start(
                        selfrows_next[:rem, wlo + full // 128, :],
                        t2sh[k][full:, :],
                    )

            def agg_layer(layer, tables, selfrows, selfrows_next=None):
                pending_ag = []
                pending_copy = []

                def issue_gather(g, ci):
                    wlo = g * GRP
                    nw = min(wlo + GRP, W) - wlo
                    nblk = nw * R
                    ixt = idxgp.tile([128, GRP * R * 8], dt.int16,
                                     tag=f"ix{ci}")
                    c0 = goff[g] + ci * nblk * 8
                    nc.sync.dma_start(
                        ixt[:, : nblk * 8],
                        idx_d[layer][:, c0 : c0 + nblk * 8],
                    )
                    gt = gbufp.tile([128, GRP * R, 128], dt.bfloat16,
                                    tag=f"gt{ci}")
                    nc.gpsimd.dma_gather(
                        gt[:, :nblk, :],
                        tables[ci][:],
                        ixt[:, : nblk * 8],
                        num_idxs=nblk * 128,
                        num_idxs_reg=nblk * 128,
                        elem_size=128,
                        elem_step=128,
                        single_packet=False,
                        queue_num=ci,
                    )
                    return gt

                # prologue: first PRE waves issued CHUNK-major, so queue k
                # starts the moment ITS table chunk is ready instead of the
                # in-order gpsimd queue stalling behind later chunks
                PRE = 3
                pre_gt = {}
                for ci in range(NCH):
                    for g in range(min(PRE, NGRP)):
                        pre_gt[(g, ci)] = issue_gather(g, ci)

                for g in range(NGRP):
                    wlo = g * GRP
                    whi = min(wlo + GRP, W)
                    nw = whi - wlo
                    nblk = nw * R          # blocks per gather call
                    gblk = NCH * nblk      # blocks per group
                    blk0 = wlo * NCH * R   # first block of group

                    drt = drgp.tile([128, GRP * NCH * R], dt.bfloat16,
                                    tag="drt")
                    nc.sync.dma_start(
                        drt[:, :gblk], dr_d[layer][:, blk0 : blk0 + gblk]
                    )
                    stw = maskp.tile([128, GRP * NCH * R, 128], dt.bfloat16,
                                     tag="stw")
                    nc.vector.tensor_tensor(
                        stw[:, :gblk, :],
                        iot[:].rearrange("p (o v) -> p o v", o=1)
                              .to_broadcast([128, gblk, 128]),
                        drt[:, :gblk].rearrange("p (b o) -> p b o", o=1)
                                     .to_broadcast([128, gblk, 128]),
                        mybir.AluOpType.is_equal,
                    )

                    gts = []
                    for ci in range(NCH):
                        if (g, ci) in pre_gt:
                            gts.append(pre_gt.pop((g, ci)))
                        else:
                            gts.append(issue_gather(g, ci))

                    # issue any pending AllGather AFTER this group's gathers
                    # so the gpsimd engine stall (waiting on t2sh writes)
                    # doesn't delay them
                    for k in pending_ag:
                        issue_ag(k, selfrows_next)
                        if k < NCH - 1:
                            pending_copy.append(k)  # t2l copy deferred
                        else:
                            nc.sync.dma_start(t2l[k][:], t2f[k][:])
                    pending_ag = []
                    # flush deferred t2l copies late in layer 1 so their
                    # HBM traffic doesn't compete with mid-layer gathers
                    if g == 21:
                        for k in pending_copy:
                            nc.sync.dma_start(t2l[k][:], t2f[k][:])
                        pending_copy = []

                    psg = psA.tile([128, GRP, 128], dt.float32, tag="psg")
                    for wi in range(wlo, whi):
                        wr = wi - wlo
                        nn = min(128, SH - wi * 128)
                        for ci in range(NCH):
                            for b in range(R):
                                blk = (ci * nw + wr) * R + b
                                nc.tensor.matmul(
                                    psg[:, wr, :],
                                    stw[:, blk, :],
                                    gts[ci][:, wr * R + b, :],
                                    start=(ci == 0 and b == 0),
                                    stop=False,
                                )
                        if has_b and layer == 1:
                            nc.tensor.matmul(
                                psg[:, wr, :],
                                clh[:, wi * 128 : (wi + 1) * 128],
                                crh[:],
                                start=False, stop=False,
                            )
                        nc.tensor.matmul(
                            psg[:, wr, :],
                            idt[:nn, :],
                            selfrows[:nn, wi, :],
                            start=False, stop=True,
                        )

                    if layer == 0:
                        # table2 rows = dinv^2 * psum, bf16, window-sharded
                        zt = zstp.tile([128, GRP, 128], dt.bfloat16, tag="zt")
                        nc.vector.tensor_tensor(
                            zt[:, :nw, :],
                            psg[:, :nw, :],
                            dw2[:, wlo:whi].rearrange("p (b o) -> p b o", o=1)
                                           .to_broadcast([128, nw, 128]),
                            mybir.AluOpType.mult,
                        )
                        for k in range(NCH):
                            lo = max(wlo, AGW[k])
                            hi = min(whi, AGW[k + 1])
                            if lo >= hi:
                                continue
                            full = AGW[k] * 128 + AG_ROWS[k]
                            r0 = lo * 128 - AGW[k] * 128
                            r1 = min(hi * 128, full) - AGW[k] * 128
                            nwk = (r1 - r0 + 127) // 128
                            wfull = (r1 - r0) // 128
                            if wfull:
                                nc.sync.dma_start(
                                    t2sh[k][r0 : r0 + wfull * 128, :]
                                    .rearrange("(w p) f -> p w f", p=128),
                                    zt[:, lo - wlo : lo - wlo + wfull, :],
                                )
                            if nwk > wfull:
                                rem = (r1 - r0) - wfull * 128
                                nc.sync.dma_start(
                                    t2sh[k][r0 + wfull * 128 : r1, :],
                                    zt[:rem, lo - wlo + wfull, :],
                                )
                        # AllGather chunk as soon as its windows are done
                        for k in range(NCH):
                            if whi == AGW[k + 1]:
                                pending_ag.append(k)
                    else:
                        for wi in range(wlo, whi):
                            wr = wi - wlo
                            nn = min(128, SH - wi * 128)
                            ot = outstp.tile([128, 128], dt.float32, tag="ot")
                            nc.scalar.activation(
                                ot[:], psg[:, wr, :],
                                mybir.ActivationFunctionType.Relu,
                                scale=dwt[:, wi : wi + 1],
                            )
                            nc.sync.dma_start(
                                out_d[wi * 128 : wi * 128 + nn, :], ot[:nn, :]
                            )
                for k in pending_ag:
                    issue_ag(k, selfrows_next)
                    nc.sync.dma_start(t2l[k][:], t2f[k][:])
                for k in pending_copy:
                    nc.sync.dma_start(t2l[k][:], t2f[k][:])

            # L2 self rows tile is filled chunk-by-chunk during layer 1
            selfrows2 = selfrp.tile([128, W, 128], dt.bfloat16,
                                    tag="selfrows")
            agg_layer(0, h1s_c, selfrows1, selfrows_next=selfrows2)
            agg_layer(1, t2l, selfrows2)

    nc.compile()
    return nc


def kernel(x, edge_index, W1, b1, W2, b2):
    x = np.asarray(x)
    meta, in_maps = _preprocess(
        x, np.asarray(edge_index), np.asarray(W1), np.asarray(b1),
        np.asarray(W2), np.asarray(b2),
    )
    nc = _build(meta)
    trace = bool(os.environ.get("KERNEL_TRACE"))
    res = bass_utils.run_bass_kernel_spmd(
        nc, in_maps, core_ids=list(range(N_CORES)), trace=trace
    )
    global last_exec_time_ns
    last_exec_time_ns = res.exec_time_ns
    out = np.empty((N, 128), np.float32)
    for i in range(N_CORES):
        out[i * SH : (i + 1) * SH, :] = res.results[i]["out"]
    return out
